# revision 10
# baseline (speedup 1.0000x reference)
"""Trainium2 Bass kernel for AttentionLayerWithMask (ragged prefix-mask attention).

Problem: B=1024, S=200, D=O=512.
  sqlen = mask.sum(1); query = proj_q(x[b, sqlen-1]); keys/values = x[b, :sqlen-1]
  out = tanh(attn @ V)

Algebraic rewrite (exact, up to fp reassociation):
  scores[b,s] = q[b] . K[b,s]   with K = x@Wk^T + bk
              = (Wk^T q[b]) . x[b,s]  + q[b].bk     (2nd term const in s -> softmax-invariant, dropped)
  out[b]      = tanh(Wv (sum_s attn[b,s] x[b,s]) + bv)   (since sum_s attn = 1)

so the big [B,S,O] K/V tensors are never materialized: per batch we need one
D-dim "folded query" qk[b] = Wk^T (Wq x_last[b] + bq), one dot per (b,s) for
scores, and one weighted sum over s for ctx.

Mapping: batch on partitions (128 per core), flash-attention-style online
softmax over s-chunks so each X chunk is loaded once and consumed for both
scores (VectorE fused affine_mul_reduce) and ctx accumulation (VectorE
scalar_tensor_tensor FMA with per-partition attn scalars). The prefix mask
makes online masking safe: valid positions come first, so the running max is
real before any fully-masked chunk appears.

Sharding: pure data parallel, batch 1024 -> 8 cores x 128.
"""

import numpy as np

B, S, D, O = 1024, 200, 512, 512
NCORES = 8
P = B // NCORES          # 128 batches per core
NK = D // 128            # 4 contraction chunks of 128
CS = 25                  # s-chunk size
NC = S // CS             # 8 chunks
NEG = -1e30

_cache = {}


def _build_nc():
    """Build + compile the Bass/Tile module (shared by all 8 cores)."""
    from contextlib import ExitStack

    import concourse.bass as bass
    import concourse.tile as tile
    from concourse import bacc, mybir
    from concourse.masks import make_identity

    f32 = mybir.dt.float32
    AF = mybir.ActivationFunctionType
    ALU = mybir.AluOpType

    nc = bacc.Bacc("TRN2", target_bir_lowering=False, debug=False, num_devices=NCORES)

    x_d = nc.dram_tensor("x", [P, S, D], f32, kind="ExternalInput").ap()
    lastT_d = nc.dram_tensor("lastT", [D, P], f32, kind="ExternalInput").ap()
    smask_d = nc.dram_tensor("smask", [P, S], f32, kind="ExternalInput").ap()
    wqT_d = nc.dram_tensor("wqT", [D, O], f32, kind="ExternalInput").ap()
    wk_d = nc.dram_tensor("wk", [O, D], f32, kind="ExternalInput").ap()
    wvT_d = nc.dram_tensor("wvT", [D, O], f32, kind="ExternalInput").ap()
    bq_d = nc.dram_tensor("bq", [NK, 128, 1], f32, kind="ExternalInput").ap()
    bv_d = nc.dram_tensor("bv", [1, O], f32, kind="ExternalInput").ap()
    out_d = nc.dram_tensor("out", [P, O], f32, kind="ExternalOutput").ap()

    with tile.TileContext(nc) as tc:
        with ExitStack() as ctx:
            consts = ctx.enter_context(tc.tile_pool(name="consts", bufs=1))
            xc_pool = ctx.enter_context(tc.tile_pool(name="xc", bufs=2))
            scr_pool = ctx.enter_context(tc.tile_pool(name="scr", bufs=2))
            st_pool = ctx.enter_context(tc.tile_pool(name="st", bufs=2))
            small_pool = ctx.enter_context(tc.tile_pool(name="small", bufs=3))
            ps_a = ctx.enter_context(tc.tile_pool(name="psA", bufs=2, space="PSUM"))
            ps_big = ctx.enter_context(tc.tile_pool(name="psBig", bufs=1, space="PSUM"))

            # ---------- constants ----------
            wqT_sb, wk_sb, wvT_sb, lastT_sb, bq_sb = [], [], [], [], []
            for k in range(NK):
                t = consts.tile([128, O], f32, tag=f"wqT{k}", name=f"wqT{k}")
                nc.sync.dma_start(t, wqT_d[k * 128:(k + 1) * 128, :])
                wqT_sb.append(t)
                t = consts.tile([128, D], f32, tag=f"wk{k}", name=f"wk{k}")
                nc.sync.dma_start(t, wk_d[k * 128:(k + 1) * 128, :])
                wk_sb.append(t)
                t = consts.tile([128, O], f32, tag=f"wvT{k}", name=f"wvT{k}")
                nc.sync.dma_start(t, wvT_d[k * 128:(k + 1) * 128, :])
                wvT_sb.append(t)
                t = consts.tile([128, P], f32, tag=f"lastT{k}", name=f"lastT{k}")
                nc.sync.dma_start(t, lastT_d[k * 128:(k + 1) * 128, :])
                lastT_sb.append(t)
                t = consts.tile([128, 1], f32, tag=f"bq{k}", name=f"bq{k}")
                nc.sync.dma_start(t, bq_d[k])
                bq_sb.append(t)
            bv_sb = consts.tile([1, O], f32, tag="bv")
            nc.sync.dma_start(bv_sb, bv_d)
            smask_sb = consts.tile([P, S], f32, tag="smask")
            nc.sync.dma_start(smask_sb, smask_d)
            ones_sb = consts.tile([1, 128], f32, tag="ones")
            nc.vector.memset(ones_sb, 1.0)
            ident = consts.tile([128, 128], f32, tag="ident")
            make_identity(nc, ident)

            # ---------- QT[o,b] = Wq @ last + bq ----------
            qt_sb = []
            for om in range(NK):
                pq = ps_a.tile([128, P], f32, tag="ptr", name=f"pq{om}")
                for kd in range(NK):
                    nc.tensor.matmul(
                        pq, lhsT=wqT_sb[kd][:, om * 128:(om + 1) * 128],
                        rhs=lastT_sb[kd], start=(kd == 0), stop=(kd == NK - 1))
                qt = consts.tile([128, P], f32, tag=f"qt{om}", name=f"qt{om}")
                nc.scalar.activation(qt, pq, AF.Identity, bias=bq_sb[om], scale=1.0)
                qt_sb.append(qt)

            # ---------- QK[b,d] = q^T Wk  (folded query, batch-major) ----------
            pqk = ps_big.tile([P, D], f32, tag="pbig", name="pqk")
            for ko in range(NK):
                nc.tensor.matmul(pqk, lhsT=qt_sb[ko], rhs=wk_sb[ko],
                                 start=(ko == 0), stop=(ko == NK - 1))
            qkb_sb = consts.tile([P, D], f32, tag="qkb")
            nc.vector.tensor_copy(qkb_sb, pqk)

            # ---------- online-softmax running state ----------
            m_run = st_pool.tile([P, 1], f32, tag="m", name="m_init")
            nc.vector.memset(m_run, -3e38)
            zacc = consts.tile([P, 1], f32, tag="zacc")
            nc.vector.memset(zacc, 0.0)
            ctx_acc = consts.tile([P, D], f32, tag="ctx")
            nc.vector.memset(ctx_acc, 0.0)

            # ---------- main loop over s-chunks ----------
            for c in range(NC):
                xc = xc_pool.tile([P, CS, D], f32, tag="xc", name=f"xc{c}")
                nc.sync.dma_start(xc, x_d[:, c * CS:(c + 1) * CS, :])

                # scores for this chunk: sc[:, si] = sum_d x[:, si, :]*qk
                sc = small_pool.tile([P, CS], f32, tag="sc", name=f"sc{c}")
                for si in range(CS):
                    t0 = scr_pool.tile([P, D], f32, tag="scr", name=f"t{c}_{si}")
                    nc.vector.affine_mul_reduce(
                        out=t0, accum_out=sc[:, si:si + 1], in0=xc[:, si, :],
                        in1=qkb_sb, scale=1.0, bias=0.0)
                nc.vector.tensor_add(sc, sc, smask_sb[:, c * CS:(c + 1) * CS])

                # online softmax update
                mc = small_pool.tile([P, 1], f32, tag="mc", name=f"mc{c}")
                nc.vector.reduce_max(mc, sc, axis=mybir.AxisListType.X)
                m_new = st_pool.tile([P, 1], f32, tag="m", name=f"m{c}")
                nc.vector.tensor_max(m_new, m_run, mc)
                nm = small_pool.tile([P, 1], f32, tag="nm", name=f"nm{c}")
                nc.vector.tensor_scalar_mul(nm, m_new, -1.0)
                alpha = small_pool.tile([P, 1], f32, tag="al", name=f"al{c}")
                nc.scalar.activation(alpha, m_run, AF.Exp, bias=nm, scale=1.0)
                pc = small_pool.tile([P, CS], f32, tag="pc", name=f"pc{c}")
                zc = small_pool.tile([P, 1], f32, tag="zc", name=f"zc{c}")
                nc.scalar.activation(pc, sc, AF.Exp, bias=nm, scale=1.0,
                                     accum_out=zc)
                # z = z*alpha + zc ; ctx *= alpha
                nc.vector.scalar_tensor_tensor(
                    out=zacc, in0=zacc, scalar=alpha, in1=zc,
                    op0=ALU.mult, op1=ALU.add)
                nc.vector.tensor_scalar_mul(ctx_acc, ctx_acc, alpha)
                m_run = m_new

                # ctx += sum_si pc[:, si] * x[:, si, :]
                for si in range(CS):
                    nc.vector.scalar_tensor_tensor(
                        out=ctx_acc, in0=xc[:, si, :], scalar=pc[:, si:si + 1],
                        in1=ctx_acc, op0=ALU.mult, op1=ALU.add)

            # ---------- normalize, project, tanh ----------
            rz = small_pool.tile([P, 1], f32, tag="rz")
            nc.vector.reciprocal(rz, zacc)
            ctx_fin = consts.tile([P, D], f32, tag="ctxf")
            nc.vector.tensor_scalar_mul(ctx_fin, ctx_acc, rz)

            ctxT_sb = []
            for kd in range(NK):
                ptk = ps_a.tile([128, P], f32, tag="ptr", name=f"ptk{kd}")
                nc.tensor.transpose(ptk, ctx_fin[:, kd * 128:(kd + 1) * 128], ident)
                t = consts.tile([128, P], f32, tag=f"ctxT{kd}", name=f"ctxT{kd}")
                nc.scalar.copy(t, ptk)
                ctxT_sb.append(t)
            pout = ps_big.tile([P, O], f32, tag="pbig", name="pout")
            for kd in range(NK):
                nc.tensor.matmul(pout, lhsT=ctxT_sb[kd], rhs=wvT_sb[kd],
                                 start=(kd == 0), stop=False)
            nc.tensor.matmul(pout, lhsT=ones_sb, rhs=bv_sb, start=False, stop=True)
            outt = consts.tile([P, O], f32, tag="outt")
            nc.scalar.activation(outt, pout, AF.Tanh)
            nc.sync.dma_start(out_d, outt)

    nc.compile()
    return nc


def _host_prep(input, mask, Wq_w, Wq_b, Wk_w, Wk_b, Wv_w, Wv_b):
    """Host-side index prep + sharding. Returns per-core input maps."""
    input = np.ascontiguousarray(input, dtype=np.float32)
    mask = np.asarray(mask)
    sqlen = mask.astype(np.int64).sum(axis=1)          # [B]
    last = input[np.arange(B), sqlen - 1]              # [B, D] gather
    valid = np.arange(S)[None, :] < (sqlen - 1)[:, None]
    smask = np.where(valid, np.float32(0.0), np.float32(NEG)).astype(np.float32)

    wqT = np.ascontiguousarray(np.asarray(Wq_w, np.float32).T)   # [D, O]
    wk = np.ascontiguousarray(np.asarray(Wk_w, np.float32))      # [O, D]
    wvT = np.ascontiguousarray(np.asarray(Wv_w, np.float32).T)   # [D, O]
    bq = np.ascontiguousarray(np.asarray(Wq_b, np.float32).reshape(NK, 128, 1))
    bv = np.ascontiguousarray(np.asarray(Wv_b, np.float32).reshape(1, O))
    # Wk_b drops out of softmax (constant shift); Wv_b enters via ones-row matmul.

    in_maps = []
    for c in range(NCORES):
        sl = slice(c * P, (c + 1) * P)
        in_maps.append({
            "x": np.ascontiguousarray(input[sl]),
            "lastT": np.ascontiguousarray(last[sl].T),
            "smask": np.ascontiguousarray(smask[sl]),
            "wqT": wqT, "wk": wk, "wvT": wvT, "bq": bq, "bv": bv,
        })
    return in_maps


def _run(in_maps, trace=False):
    from concourse.bass_utils import run_bass_kernel_spmd
    if "nc" not in _cache:
        _cache["nc"] = _build_nc()
    res = run_bass_kernel_spmd(_cache["nc"], in_maps, list(range(NCORES)),
                               trace=trace)
    return res


def kernel(input, mask, Wq_w, Wq_b, Wk_w, Wk_b, Wv_w, Wv_b):
    in_maps = _host_prep(input, mask, Wq_w, Wq_b, Wk_w, Wk_b, Wv_w, Wv_b)
    res = _run(in_maps, trace=False)
    out = np.concatenate([r["out"] for r in res.results], axis=0)
    return out
